# revision 2
# baseline (speedup 1.0000x reference)
"""Trainium2 Bass kernel for nn_CustomParameterTransform (scatter_memory).

Reference semantics: coord_v [256, 30] holds 10 (x, y, mass) triplets per
sample. Each triplet maps to integer grid indices (x_i, y_i, m_i); a one-hot
volume z [B, 16, 128, 128] is scattered (z[b, m, y, x] = 1) and the output is
concat(1-z, z) over the channel axis -> [256, 32, 128, 128] f32 (512 MB).

Strategy (8 NeuronCores, batch-sharded, no cross-core comm):
  - The output is almost entirely constant: the first 16 channels are 1.0
    except at scatter points, the last 16 are 0.0 except at scatter points.
    The whole chip is HBM-write-bound (~2.8 TB/s aggregate for the 8 cores,
    ~410 GB/s/core), so the only controllables are the ramp before the first
    fill, the post-fill tail, and instruction count (the event-lowered
    epilogue clears every event semaphore one by one).
  - Per core (32 samples, 64 MB slab): fill from two small constant SBUF
    tiles (ones/zeros [128, 4096], ~1.8 us memsets split across two engines)
    via 16 strided fills -- each fill writes the ones-halves (or zeros-
    halves) of 4 consecutive slabs through a [[SLAB,4],[1,HALF]] DRAM view,
    reading the tile twice through a stride-0 repeat dim. Fills start ~2 us
    into the kernel (the stock const-AP all-engine barrier in Bass.__init__
    is patched out; nothing in this kernel uses const_aps, and NEFF
    re-execution is already gated on all engines having ended).
  - The 640 scatter points are fixed up with indirect (scatter) DMAs whose
    deps are wired to just the fills covering their samples, so all but the
    last column overlap the fill phase.
  - Indices are computed on the host with the exact same jax ops as the
    reference (bit-identical floor/log10 behavior) and passed per-core as a
    [128, 5] int32 tensor of flat element offsets.
"""

import numpy as np

B = 256
NSRC = 10
NMC = 16
L = 128
NCORES = 8
BL = B // NCORES          # 32 samples per core
PLANE = L * L             # 16384
HALF = NMC * PLANE        # 262144 elements per half-slab
SLAB = 2 * HALF           # 524288 elements per sample
OUT_ELEMS = BL * SLAB     # 16777216 per core (64 MB)

N_SCATTER_COLS = 5        # 640 scatter writes = 128 partitions x 5 columns
SLABS_PER_FILL = 4        # each fill covers 4 slabs' ones- or zeros-halves
N_FILLS = BL // SLABS_PER_FILL  # 8 fills per kind

_CACHE = {}


def _build_nc():
    import concourse.bass as bass
    import concourse.tile as tile
    from concourse import bacc, mybir
    from concourse.tile_rust import add_dep_helper

    import types as _types
    from concourse.vector_clock import ScopedClock

    # The const-AP registration in Bass.__init__ ends with an all-engine
    # barrier (~4.5 us of event-sem chaining at the head of every
    # execution). This kernel never touches const_aps -- memset packs its
    # immediate and the DMAs don't use them -- so elide the barrier for
    # the duration of construction.
    _orig_barrier = bass.Bass.all_engine_barrier
    bass.Bass.all_engine_barrier = lambda self, **kw: None
    try:
        nc = bacc.Bacc("TRN2", target_bir_lowering=False, debug=False,
                       num_devices=NCORES)
    finally:
        bass.Bass.all_engine_barrier = _orig_barrier

    def _light_drain_and_barrier(self, tick_clock, wait_clock):
        """Replaces TileContext._drain_and_barrier for this kernel. The
        stock epilogue is drain + two all-engine EVSEM butterfly barriers
        around the sem clear (~9 us after event lowering). Requirements at
        kernel end are: (1) all DMA completions observed, (2) sems cleared
        for NEFF re-execution, (3) the clear happens after every engine's
        last sem use. (1) is the sync drain's global-clock waits; (3) is a
        counting-sem join (sync arrives only after the drain, so join>=4
        implies all DMA done); (2) is the ranged clear. The second barrier
        is unnecessary: a re-execution cannot start until every engine --
        including the clearing gpsimd -- has ended."""
        nc_ = self.nc
        drain_inst = nc_.sync.drain()
        wait_clock.add_sem_waits(
            drain_inst.ins, ScopedClock({None: tick_clock.global_clock}))
        join = nc_.alloc_semaphore("tail_join")
        for eng in nc_.engines.values():
            if eng is not nc_.gpsimd:
                eng.sem_inc(join, 1)
        n_other = len(nc_.engines) - 1
        nc_.gpsimd.wait_ge(join, n_other)
        popped = nc_._tile_sem_poison_stack.pop()
        assert popped is self._sem_poison
        sems = list(self.sems.allocated().values())
        nc_.clear_and_free_semaphores(sems + [join])

    offs = nc.dram_tensor("offs", [128, N_SCATTER_COLS], mybir.dt.int32,
                          kind="ExternalInput").ap()
    out = nc.dram_tensor("out", [BL, SLAB], mybir.dt.float32,
                         kind="ExternalOutput").ap()

    with tile.TileContext(nc) as tc:
        tc._drain_and_barrier = _types.MethodType(_light_drain_and_barrier, tc)
        with tc.tile_pool(name="src", bufs=1) as src_pool, \
             tc.tile_pool(name="small", bufs=1) as small_pool:
            # Constant source tiles. Memset costs ~0.86 ns/column on any of
            # vector/gpsimd/scalar regardless of channel count, so each
            # tile's columns are split across vector+gpsimd: ones_t ready
            # ~2 us, zeros_t ~3.8 us.
            ones_t = src_pool.tile([128, 4096], mybir.dt.float32)
            zeros_t = src_pool.tile([128, 4096], mybir.dt.float32)
            nc.vector.memset(ones_t[:, 0:2048], 1.0)
            nc.gpsimd.memset(ones_t[:, 2048:4096], 1.0)
            nc.vector.memset(zeros_t[:, 0:2048], 0.0)
            nc.gpsimd.memset(zeros_t[:, 2048:4096], 0.0)

            # Scatter offsets: [128, 5] int32 flat element indices.
            # Column layout (entries are points p = 10*s + k, in order):
            #   col 0: ones-half offsets for points   0..127 (samples  0-12)
            #   col 1: z-half    offsets for points   0..127 (samples  0-12)
            #   col 2: ones-half offsets for points 128..255 (samples 12-25)
            #   col 3: z-half    offsets for points 128..255 (samples 12-25)
            #   col 4: rows 0-63 ones-half pts 256..319, rows 64-127 z-half
            #          pts 256..319 (samples 25-31)
            # offs load and vals memsets are only needed by the scatters
            # (earliest ~45 us in) -- keep them after the source memsets so
            # they don't delay the first fills.
            offs_t = small_pool.tile([128, N_SCATTER_COLS], mybir.dt.int32)
            nc.gpsimd.dma_start(offs_t[:, :], offs[:, :])
            vals_t = small_pool.tile([128, N_SCATTER_COLS], mybir.dt.float32)
            nc.gpsimd.memset(vals_t[:, 0:1], 0.0)
            nc.gpsimd.memset(vals_t[:, 1:2], 1.0)
            nc.gpsimd.memset(vals_t[:, 2:3], 0.0)
            nc.gpsimd.memset(vals_t[:, 3:4], 1.0)
            nc.gpsimd.memset(vals_t[0:64, 4:5], 0.0)
            nc.gpsimd.memset(vals_t[64:128, 4:5], 1.0)

            # Source AP: read the 2 MB tile SLABS_PER_FILL/2 times through a
            # stride-0 repeat dim placed after the partition dim (values are
            # constant, so the exact element correspondence is irrelevant).
            rep = SLABS_PER_FILL // 2
            ones_src = ones_t[:, :].unsqueeze(1).broadcast_to([128, rep, 4096])
            zeros_src = zeros_t[:, :].unsqueeze(1).broadcast_to([128, rep, 4096])

            # Fill k of a kind writes the ones-/zeros-halves of slabs
            # 4k..4k+3 through a strided [[SLAB,4],[1,HALF]] view. The two
            # HWDGE rings (sync, scalar) alternate fills; both rings open
            # with a ones fill so they can start as soon as ones_t is ready.
            ones_fills = []
            zeros_fills = []
            for k in range(N_FILLS):
                s0 = k * SLABS_PER_FILL
                s1 = s0 + SLABS_PER_FILL
                eng = nc.sync if k % 2 == 0 else nc.scalar
                ones_fills.append(
                    eng.dma_start(out[s0:s1, 0:HALF], ones_src))
                zeros_fills.append(
                    eng.dma_start(out[s0:s1, HALF:SLAB], zeros_src))

            # Which fills each scatter column must wait for (fill k covers
            # samples 4k..4k+3).
            col_deps = [
                ones_fills[0:4],               # col 0: samples 0-12
                zeros_fills[0:4],              # col 1
                ones_fills[3:7],               # col 2: samples 12-25
                zeros_fills[3:7],              # col 3
                ones_fills[6:8] + zeros_fills[6:8],  # col 4: samples 25-31
            ]

            # Narrow declared out AP ([1, 1] at offset 0): the real write
            # addresses come from the offset tensor; a full-tensor AP would
            # make Tile serialize every scatter behind every fill (WAW), and
            # the explicit col_deps edges below provide the true ordering.
            out2d = out[0:1, 0:1]
            for j in range(N_SCATTER_COLS):
                sc = nc.gpsimd.indirect_dma_start(
                    out=out2d,
                    out_offset=bass.IndirectOffsetOnAxis(
                        ap=offs_t[:, j:j + 1], axis=0),
                    in_=vals_t[:, j:j + 1],
                    in_offset=None,
                )
                for fl in col_deps[j]:
                    add_dep_helper(sc.ins, fl.ins,
                                   reason="scatter after its sample fills")

    nc.compile()
    return nc


def _compute_indices(coord_v, lows, highs, nmc, L_):
    """Replicates reference.py lines exactly (same jax ops on the default
    device) so the floor/log10 bin boundaries match bit-for-bit."""
    import jax.numpy as jnp

    cv = jnp.asarray(np.asarray(coord_v, dtype=np.float32))
    n = cv.shape[1] // 3
    v10 = cv.at[:, 2::3].set(jnp.log10(cv[:, 2::3]))
    lo = jnp.tile(jnp.asarray(np.asarray(lows, dtype=np.float32)), n)
    hi = jnp.tile(jnp.asarray(np.asarray(highs, dtype=np.float32)), n)
    coord_grid = (v10 - lo) / (hi - lo)
    tr = coord_grid.reshape(-1, 3)
    x_i = jnp.floor(tr[:, 0] * L_).astype(jnp.int32)
    y_i = jnp.floor(tr[:, 1] * L_).astype(jnp.int32)
    m_i = jnp.floor(tr[:, 2] * nmc).astype(jnp.int32)
    return (np.asarray(x_i), np.asarray(y_i), np.asarray(m_i))


def _prepare_in_maps(coord_v, lows, highs, nmc, L):
    nmc = int(nmc)
    L_ = int(L)
    x_i, y_i, m_i = _compute_indices(coord_v, lows, highs, nmc, L_)
    n_batch = coord_v.shape[0]
    n = coord_v.shape[1] // 3
    b_i = np.repeat(np.arange(n_batch, dtype=np.int64), n)

    # Flat element offsets (per core, local slab coordinates).
    flat_ones = ((b_i % BL) * SLAB + m_i.astype(np.int64) * PLANE
                 + y_i.astype(np.int64) * L_ + x_i.astype(np.int64))
    flat_z = flat_ones + HALF

    in_maps = []
    pts_per_core = BL * n  # 320
    for c in range(NCORES):
        sel = slice(c * pts_per_core, (c + 1) * pts_per_core)
        po = flat_ones[sel]
        pz = flat_z[sel]
        offs_np = np.zeros((128, N_SCATTER_COLS), dtype=np.int32)
        offs_np[:, 0] = po[0:128]
        offs_np[:, 1] = pz[0:128]
        offs_np[:, 2] = po[128:256]
        offs_np[:, 3] = pz[128:256]
        offs_np[0:64, 4] = po[256:320]
        offs_np[64:128, 4] = pz[256:320]
        in_maps.append({"offs": offs_np})
    return in_maps


def _run(in_maps, **kwargs):
    if "nc" not in _CACHE:
        _CACHE["nc"] = _build_nc()
    nc = _CACHE["nc"]
    from concourse.bass_utils import run_bass_kernel_spmd
    return run_bass_kernel_spmd(nc, in_maps, core_ids=list(range(NCORES)),
                                **kwargs)


def kernel(coord_v, lows, highs, nmc, L):
    nmc = int(nmc)
    L_ = int(L)
    assert nmc == NMC and L_ == globals()["L"], (nmc, L_)

    in_maps = _prepare_in_maps(coord_v, lows, highs, nmc, L_)
    res = _run(in_maps)
    parts = [res.results[c]["out"].reshape(BL, 2 * NMC, L_, L_)
             for c in range(NCORES)]
    return np.concatenate(parts, axis=0)


# revision 6
# speedup vs baseline: 3.8209x; 3.8209x over previous
"""Trainium2 Bass kernel for nn_CustomParameterTransform (scatter_memory).

Reference semantics: coord_v [256, 30] holds 10 (x, y, mass) triplets per
sample. Each triplet maps to integer grid indices (x_i, y_i, m_i); a one-hot
volume z [B, 16, 128, 128] is scattered (z[b, m, y, x] = 1) and the output is
concat(1-z, z) over the channel axis -> [256, 32, 128, 128] f32 (512 MB).

Strategy (8 NeuronCores, batch-sharded, no cross-core comm):
  - The output is almost entirely constant: the first 16 channels are 1.0
    except at scatter points, the last 16 are 0.0 except at scatter points.
    The whole chip is HBM-write-bound (~2.8 TB/s aggregate for the 8 cores,
    ~410 GB/s/core), so the only controllables are the ramp before the first
    fill, the post-fill tail, and instruction count (the event-lowered
    epilogue clears every event semaphore one by one).
  - Per core (32 samples, 64 MB slab): fill from one [128, 4096] SBUF tile
    whose rows 0-63 are 1.0 and rows 64-127 are 0.0 -- one full sweep of
    the tile in partition-major order is exactly one slab (1 MB of ones
    then 1 MB of zeros), so every fill is a plain contiguous 2 MB write.
    Both DMA sides stay 2-D (SBUF [128 x 4096], DRAM [512 x 4096]), which
    keeps the HWDGE PDMA2D fast path; 3-D/strided dest APs or stride-0
    repeat sources demote the DMA to an engine-sequenced slow path that
    runs ~5x slower (measured). Memset cost is per-column (~0.86 ns/col
    regardless of rows), so the row-split tile costs the same ~3.6 us to
    initialize as a plain constant tile, split across vector+gpsimd.
    Fills start ~2 us into the kernel (the stock const-AP all-engine
    barrier in Bass.__init__ is patched out; nothing in this kernel uses
    const_aps, and NEFF re-execution is already gated on all engines
    having ended).
  - The 640 scatter points are fixed up with indirect (scatter) DMAs whose
    deps are wired to just the fills covering their samples, so all but the
    last column overlap the fill phase.
  - Indices are computed on the host with the exact same jax ops as the
    reference (bit-identical floor/log10 behavior) and passed per-core as a
    [128, 5] int32 tensor of flat element offsets.
"""

import numpy as np

B = 256
NSRC = 10
NMC = 16
L = 128
NCORES = 8
BL = B // NCORES          # 32 samples per core
PLANE = L * L             # 16384
HALF = NMC * PLANE        # 262144 elements per half-slab
SLAB = 2 * HALF           # 524288 elements per sample
OUT_ELEMS = BL * SLAB     # 16777216 per core (64 MB)

N_SCATTER_COLS = 5        # 640 scatter writes = 128 partitions x 5 columns

_CACHE = {}


def _build_nc():
    import concourse.bass as bass
    import concourse.tile as tile
    from concourse import bacc, mybir
    from concourse.tile_rust import add_dep_helper

    import types as _types
    from concourse.vector_clock import ScopedClock

    # The const-AP registration in Bass.__init__ ends with an all-engine
    # barrier (~4.5 us of event-sem chaining at the head of every
    # execution). This kernel never touches const_aps -- memset packs its
    # immediate and the DMAs don't use them -- so elide the barrier for
    # the duration of construction.
    _orig_barrier = bass.Bass.all_engine_barrier
    bass.Bass.all_engine_barrier = lambda self, **kw: None
    try:
        nc = bacc.Bacc("TRN2", target_bir_lowering=False, debug=False,
                       num_devices=NCORES)
    finally:
        bass.Bass.all_engine_barrier = _orig_barrier

    def _light_drain_and_barrier(self, tick_clock, wait_clock):
        """Replaces TileContext._drain_and_barrier for this kernel. The
        stock epilogue is drain + two all-engine EVSEM butterfly barriers
        around the sem clear (~9 us after event lowering). Requirements at
        kernel end are: (1) all DMA completions observed, (2) sems cleared
        for NEFF re-execution, (3) the clear happens after every engine's
        last sem use. (1) is the sync drain's global-clock waits; (3) is a
        counting-sem join (sync arrives only after the drain, so join>=4
        implies all DMA done); (2) is the ranged clear. The second barrier
        is unnecessary: a re-execution cannot start until every engine --
        including the clearing gpsimd -- has ended."""
        nc_ = self.nc
        drain_inst = nc_.sync.drain()
        wait_clock.add_sem_waits(
            drain_inst.ins, ScopedClock({None: tick_clock.global_clock}))
        join = nc_.alloc_semaphore("tail_join")
        for eng in nc_.engines.values():
            if eng is not nc_.gpsimd:
                eng.sem_inc(join, 1)
        n_other = len(nc_.engines) - 1
        nc_.gpsimd.wait_ge(join, n_other)
        popped = nc_._tile_sem_poison_stack.pop()
        assert popped is self._sem_poison
        sems = list(self.sems.allocated().values())
        nc_.clear_and_free_semaphores(sems + [join])

    offs = nc.dram_tensor("offs", [128, N_SCATTER_COLS], mybir.dt.int32,
                          kind="ExternalInput").ap()
    out = nc.dram_tensor("out", [BL, SLAB], mybir.dt.float32,
                         kind="ExternalOutput").ap()

    with tile.TileContext(nc) as tc:
        tc._drain_and_barrier = _types.MethodType(_light_drain_and_barrier, tc)
        with tc.tile_pool(name="src", bufs=1) as src_pool, \
             tc.tile_pool(name="small", bufs=1) as small_pool:
            # Slab-image source tile: rows 0-63 ones (= the 1 MB ones half),
            # rows 64-127 zeros. Columns split across vector+gpsimd, ones
            # rows first: the ones half is ready ~2 us in, the full tile
            # ~3.8 us.
            slab_t = src_pool.tile([128, 4096], mybir.dt.float32)
            nc.vector.memset(slab_t[0:64, 0:2048], 1.0)
            nc.gpsimd.memset(slab_t[0:64, 2048:4096], 1.0)
            nc.vector.memset(slab_t[64:128, 0:2048], 0.0)
            nc.gpsimd.memset(slab_t[64:128, 2048:4096], 0.0)

            # Scatter offsets: [128, 5] int32 flat element indices.
            # Column layout (entries are points p = 10*s + k, in order):
            #   col 0: ones-half offsets for points   0..127 (samples  0-12)
            #   col 1: z-half    offsets for points   0..127 (samples  0-12)
            #   col 2: ones-half offsets for points 128..255 (samples 12-25)
            #   col 3: z-half    offsets for points 128..255 (samples 12-25)
            #   col 4: rows 0-63 ones-half pts 256..319, rows 64-127 z-half
            #          pts 256..319 (samples 25-31)
            # offs load and vals memsets are only needed by the scatters
            # (earliest ~45 us in) -- keep them after the source memsets so
            # they don't delay the first fills.
            offs_t = small_pool.tile([128, N_SCATTER_COLS], mybir.dt.int32)
            nc.gpsimd.dma_start(offs_t[:, :], offs[:, :])
            vals_t = small_pool.tile([128, N_SCATTER_COLS], mybir.dt.float32)
            nc.gpsimd.memset(vals_t[:, 0:1], 0.0)
            nc.gpsimd.memset(vals_t[:, 1:2], 1.0)
            nc.gpsimd.memset(vals_t[:, 2:3], 0.0)
            nc.gpsimd.memset(vals_t[:, 3:4], 1.0)
            nc.gpsimd.memset(vals_t[0:64, 4:5], 0.0)
            nc.gpsimd.memset(vals_t[64:128, 4:5], 1.0)

            # Fills. Slabs 0-1 are filled as four 1 MB half-fills so the
            # rings can start as soon as the ones rows are ready (~2 us),
            # before the zeros rows finish; the remaining 30 slabs are one
            # contiguous 2 MB fill each, alternating between the two HWDGE
            # rings (sync, scalar). sample_fills[s] lists the fills that
            # write slab s.
            sample_fills = {}
            for s in (0, 1):
                eng = nc.sync if s == 0 else nc.scalar
                f_ones = eng.dma_start(out[s:s + 1, 0:HALF],
                                       slab_t[0:64, :])
                f_zeros = eng.dma_start(out[s:s + 1, HALF:SLAB],
                                        slab_t[64:128, :])
                sample_fills[s] = [f_ones, f_zeros]
            for s in range(2, BL):
                eng = nc.sync if s % 2 == 0 else nc.scalar
                sample_fills[s] = [
                    eng.dma_start(out[s:s + 1, :], slab_t[:, :])]

            # Which fills each scatter column must wait for.
            def deps(lo, hi):
                return [f for s in range(lo, hi) for f in sample_fills[s]]
            col_deps = [
                deps(0, 13),    # col 0: samples 0-12
                deps(0, 13),    # col 1
                deps(12, 26),   # col 2: samples 12-25
                deps(12, 26),   # col 3
                deps(25, BL),   # col 4: samples 25-31
            ]

            # Narrow declared out AP ([1, 1] at offset 0): the real write
            # addresses come from the offset tensor; a full-tensor AP would
            # make Tile serialize every scatter behind every fill (WAW), and
            # the explicit col_deps edges below provide the true ordering.
            out2d = out[0:1, 0:1]
            for j in range(N_SCATTER_COLS):
                sc = nc.gpsimd.indirect_dma_start(
                    out=out2d,
                    out_offset=bass.IndirectOffsetOnAxis(
                        ap=offs_t[:, j:j + 1], axis=0),
                    in_=vals_t[:, j:j + 1],
                    in_offset=None,
                )
                for fl in col_deps[j]:
                    add_dep_helper(sc.ins, fl.ins,
                                   reason="scatter after its sample fills")

    nc.compile()
    return nc


def _compute_indices(coord_v, lows, highs, nmc, L_):
    """Replicates reference.py lines exactly (same jax ops on the default
    device) so the floor/log10 bin boundaries match bit-for-bit."""
    import jax.numpy as jnp

    cv = jnp.asarray(np.asarray(coord_v, dtype=np.float32))
    n = cv.shape[1] // 3
    v10 = cv.at[:, 2::3].set(jnp.log10(cv[:, 2::3]))
    lo = jnp.tile(jnp.asarray(np.asarray(lows, dtype=np.float32)), n)
    hi = jnp.tile(jnp.asarray(np.asarray(highs, dtype=np.float32)), n)
    coord_grid = (v10 - lo) / (hi - lo)
    tr = coord_grid.reshape(-1, 3)
    x_i = jnp.floor(tr[:, 0] * L_).astype(jnp.int32)
    y_i = jnp.floor(tr[:, 1] * L_).astype(jnp.int32)
    m_i = jnp.floor(tr[:, 2] * nmc).astype(jnp.int32)
    return (np.asarray(x_i), np.asarray(y_i), np.asarray(m_i))


def _prepare_in_maps(coord_v, lows, highs, nmc, L):
    nmc = int(nmc)
    L_ = int(L)
    x_i, y_i, m_i = _compute_indices(coord_v, lows, highs, nmc, L_)
    n_batch = coord_v.shape[0]
    n = coord_v.shape[1] // 3
    b_i = np.repeat(np.arange(n_batch, dtype=np.int64), n)

    # Flat element offsets (per core, local slab coordinates).
    flat_ones = ((b_i % BL) * SLAB + m_i.astype(np.int64) * PLANE
                 + y_i.astype(np.int64) * L_ + x_i.astype(np.int64))
    flat_z = flat_ones + HALF

    in_maps = []
    pts_per_core = BL * n  # 320
    for c in range(NCORES):
        sel = slice(c * pts_per_core, (c + 1) * pts_per_core)
        po = flat_ones[sel]
        pz = flat_z[sel]
        offs_np = np.zeros((128, N_SCATTER_COLS), dtype=np.int32)
        offs_np[:, 0] = po[0:128]
        offs_np[:, 1] = pz[0:128]
        offs_np[:, 2] = po[128:256]
        offs_np[:, 3] = pz[128:256]
        offs_np[0:64, 4] = po[256:320]
        offs_np[64:128, 4] = pz[256:320]
        in_maps.append({"offs": offs_np})
    return in_maps


def _run(in_maps, **kwargs):
    if "nc" not in _CACHE:
        _CACHE["nc"] = _build_nc()
    nc = _CACHE["nc"]
    from concourse.bass_utils import run_bass_kernel_spmd
    return run_bass_kernel_spmd(nc, in_maps, core_ids=list(range(NCORES)),
                                **kwargs)


def kernel(coord_v, lows, highs, nmc, L):
    nmc = int(nmc)
    L_ = int(L)
    assert nmc == NMC and L_ == globals()["L"], (nmc, L_)

    in_maps = _prepare_in_maps(coord_v, lows, highs, nmc, L_)
    res = _run(in_maps)
    parts = [res.results[c]["out"].reshape(BL, 2 * NMC, L_, L_)
             for c in range(NCORES)]
    return np.concatenate(parts, axis=0)
